# revision 4
# baseline (speedup 1.0000x reference)
"""Multi-head causal self-attention on 8 Trainium2 NeuronCores.

Sharding: 4-way data parallel over batch x 2-way tensor parallel over heads.
Core c handles batch c//2 and head group c%2 (8 of 16 heads). Each core
computes QKV projections for its head group, causal attention, and a partial
output projection (row-split Wo); the two partials per batch are summed on
the host. Bias is added on-device by the g=0 core (g=1 gets zeros).

Schedule (all bf16 matmuls, fp32 PSUM):
- Heads are processed in PAIRS (2j, 2j+1). The two K^T.Q score matmuls have
  contraction 64, so they run concurrently in PE row-groups 0/64, writing the
  two halves of one [128,1024] PSUM tile. One exp (scalar engine) covers both.
- V tiles carry a ones column, so the ctx matmul also yields the softmax
  denominators; each pair is normalized right after its ctx accumulation
  finishes (tiny [1,512] reciprocals), off the PE critical path.
- Query dim is walked in 512-wide chunks; each chunk's out-projection and the
  next t-block's QKV chains are emitted BETWEEN attention pairs so the PE
  queue never drains while the scalar engine works through the exp stream
  (keeps the PE HAM-warm at 2.4 GHz).
"""

import numpy as np

B, T, D = 4, 2048, 1024
HEADS = 16
N_CORES = 8
HPC = 8               # heads per core
HD = HPC * 64         # 512, per-core projection width
NT_D = D // 128       # 8 d-tiles
TB = 512              # t-block width for QKV streaming
NT_TB = T // TB       # 4
NT_T = T // 128       # 16 t-tiles
CHUNK = 512           # attention q-chunk width
N_CHUNK = T // CHUNK  # 4
NPAIR = HPC // 2      # 4 head pairs (pair j <-> hdt j)

_NC = None


def _build():
    import concourse.tile as tile
    import concourse.mybir as mybir
    from concourse import bacc
    from contextlib import ExitStack

    F32 = mybir.dt.float32
    BF16 = mybir.dt.bfloat16
    EXP = mybir.ActivationFunctionType.Exp

    nc = bacc.Bacc("TRN2", target_bir_lowering=False, debug=False,
                   num_devices=N_CORES)

    xT_ext = nc.dram_tensor("xT", [D, T], BF16, kind="ExternalInput")
    wqT_ext = nc.dram_tensor("wqT", [D, HD], BF16, kind="ExternalInput")
    wkT_ext = nc.dram_tensor("wkT", [D, HD], BF16, kind="ExternalInput")
    wvT_ext = nc.dram_tensor("wvT", [D, HD], BF16, kind="ExternalInput")
    woT_ext = nc.dram_tensor("woT", [HD, D], BF16, kind="ExternalInput")
    bias_ext = nc.dram_tensor("bias", [1, D], F32, kind="ExternalInput")
    mask_ext = nc.dram_tensor("mask", [128, 128], BF16, kind="ExternalInput")
    out_ext = nc.dram_tensor("out", [T, D], F32, kind="ExternalOutput")

    with tile.TileContext(nc) as tc, ExitStack() as ctx:
        # ---- pools (PSUM: 4 + 2 + 2 = 8 banks) ------------------------
        wqkv_pool = ctx.enter_context(tc.tile_pool(name="wqkv", bufs=1))
        wo_pool = ctx.enter_context(tc.tile_pool(name="wo", bufs=1))
        qk_pool = ctx.enter_context(tc.tile_pool(name="qk", bufs=1))
        v_pool = ctx.enter_context(tc.tile_pool(name="v", bufs=1))
        xt_pool = ctx.enter_context(tc.tile_pool(name="xt", bufs=2))
        small = ctx.enter_context(tc.tile_pool(name="small", bufs=1))
        pt_pool = ctx.enter_context(tc.tile_pool(name="pt", bufs=3))
        ctxT_pool = ctx.enter_context(tc.tile_pool(name="ctxT", bufs=2))
        norm_pool = ctx.enter_context(tc.tile_pool(name="norm", bufs=2))
        out_pool = ctx.enter_context(tc.tile_pool(name="outsb", bufs=3))
        s_ps_pool = ctx.enter_context(
            tc.tile_pool(name="sps", bufs=2, space="PSUM"))
        ctx_ps_pool = ctx.enter_context(
            tc.tile_pool(name="ctxps", bufs=1, space="PSUM"))
        proj_ps = ctx.enter_context(
            tc.tile_pool(name="projps", bufs=2, space="PSUM"))

        # ---- static SBUF tensors --------------------------------------
        QT = [qk_pool.tile([128, T], BF16, tag=f"QT{i}", name=f"QT{i}")
              for i in range(NPAIR)]
        KT = [qk_pool.tile([128, T], BF16, tag=f"KT{i}", name=f"KT{i}")
              for i in range(NPAIR)]
        V = [v_pool.tile([128, HPC * 65], BF16, tag=f"V{i}", name=f"V{i}")
             for i in range(NT_T)]

        w_sb = {}

        def load_weights_qkv():
            for name, ext in (("q", wqT_ext), ("k", wkT_ext), ("v", wvT_ext)):
                tiles = []
                for dt in range(NT_D):
                    t_ = wqkv_pool.tile([128, HD], BF16, tag=f"w{name}{dt}",
                                        name=f"w{name}{dt}")
                    nc.sync.dma_start(t_[:], ext[dt * 128:(dt + 1) * 128, :])
                    tiles.append(t_)
                w_sb[name] = tiles

        def load_weights_wo():
            wo_tiles = []
            for hdt in range(NPAIR):
                t_ = wo_pool.tile([128, D], BF16, tag=f"wo{hdt}",
                                  name=f"wo{hdt}")
                nc.sync.dma_start(t_[:], woT_ext[hdt * 128:(hdt + 1) * 128, :])
                wo_tiles.append(t_)
            w_sb["o"] = wo_tiles

        xts_all = {}

        def load_x(tb):
            xts = []
            for dt in range(NT_D):
                t_ = xt_pool.tile([128, TB], BF16, tag=f"xt{dt}",
                                  name=f"xt{dt}")
                nc.sync.dma_start(
                    t_[:], xT_ext[dt * 128:(dt + 1) * 128,
                                  tb * TB:(tb + 1) * TB])
                xts.append(t_)
            xts_all[tb] = xts

        # one QKV "unit" = one accumulation chain (8 matmuls) + drain copy
        def qk_unit(tb, wname, dst, hdt):
            xts = xts_all[tb]
            ps = proj_ps.tile([128, TB], F32, tag="proj", name="projps")
            for dt in range(NT_D):
                nc.tensor.matmul(
                    ps[:],
                    w_sb[wname][dt][:, hdt * 128:(hdt + 1) * 128],
                    xts[dt][:],
                    start=(dt == 0), stop=(dt == NT_D - 1))
            nc.vector.tensor_copy(
                dst[hdt][:, tb * TB:(tb + 1) * TB], ps[:])

        def v_unit(tb, j):
            xts = xts_all[tb]
            tt = tb * (TB // 128) + j
            ps = proj_ps.tile([128, HD], F32, tag="proj", name="projps")
            for dt in range(NT_D):
                nc.tensor.matmul(
                    ps[:],
                    xts[dt][:, j * 128:(j + 1) * 128],
                    w_sb["v"][dt][:],
                    start=(dt == 0), stop=(dt == NT_D - 1))
            v3 = V[tt][:].rearrange("p (h c) -> p h c", c=65)
            nc.vector.memset(v3[:, :, 64:65], 1.0)
            nc.vector.tensor_copy(
                v3[:, :, 0:64],
                ps[:].rearrange("p (h c) -> p h c", c=64))

        def qkv_units(tb):
            us = []
            for hdt in range(NPAIR):
                us.append(lambda tb=tb, hdt=hdt: qk_unit(tb, "q", QT, hdt))
            for hdt in range(NPAIR):
                us.append(lambda tb=tb, hdt=hdt: qk_unit(tb, "k", KT, hdt))
            for j in range(TB // 128):
                us.append(lambda tb=tb, j=j: v_unit(tb, j))
            return us

        # ---- attention: one head pair x one q-chunk -------------------
        ctxT = {}  # (c % 2, j) -> tile
        LAG = 2    # ctx matmuls trail the S/exp stream by this many kt

        def pair_attention(j, c, filler=None):
            q0 = c * CHUNK
            kt_max = 4 * c + 3
            h0, h1 = 2 * j, 2 * j + 1
            ctx01 = ctx_ps_pool.tile([65, 2 * CHUNK], F32, tag="ctx",
                                     name="ctxps")
            pending = []

            def emit_ctx(kt, pt):
                nc.tensor.matmul(
                    ctx01[:, 0:CHUNK],
                    V[kt][:, h0 * 65:(h0 + 1) * 65],
                    pt[:, 0:CHUNK],
                    start=(kt == 0), stop=(kt == kt_max))
                nc.tensor.matmul(
                    ctx01[:, CHUNK:2 * CHUNK],
                    V[kt][:, h1 * 65:(h1 + 1) * 65],
                    pt[:, CHUNK:2 * CHUNK],
                    start=(kt == 0), stop=(kt == kt_max))

            for kt in range(kt_max + 1):
                off = max(0, kt * 128 - q0)
                s01 = s_ps_pool.tile([128, 2 * CHUNK], F32, tag="s",
                                     name="sps")
                nc.tensor.matmul(
                    s01[:, 0:CHUNK],
                    KT[j][0:64, kt * 128:(kt + 1) * 128],
                    QT[j][0:64, q0:q0 + CHUNK],
                    start=True, stop=True)
                nc.tensor.matmul(
                    s01[:, CHUNK:2 * CHUNK],
                    KT[j][64:128, kt * 128:(kt + 1) * 128],
                    QT[j][64:128, q0:q0 + CHUNK],
                    start=True, stop=True)
                pt = pt_pool.tile([128, 2 * CHUNK], BF16, tag="pt", name="pt")
                nc.scalar.activation(pt[:, off:2 * CHUNK],
                                     s01[:, off:2 * CHUNK], EXP, scale=0.125)
                if off > 0:
                    nc.vector.memset(pt[:, 0:off], 0.0)
                    nc.vector.memset(pt[:, CHUNK:CHUNK + off], 0.0)
                if kt * 128 >= q0:
                    nc.vector.tensor_mul(pt[:, off:off + 128],
                                         pt[:, off:off + 128], mask_sb[:])
                    nc.vector.tensor_mul(
                        pt[:, CHUNK + off:CHUNK + off + 128],
                        pt[:, CHUNK + off:CHUNK + off + 128], mask_sb[:])
                pending.append((kt, pt))
                if len(pending) > LAG:
                    emit_ctx(*pending.pop(0))
                if filler is not None:
                    filler.tick()
            for item in pending:
                emit_ctx(*item)

            # normalize pair into bf16 ctxT tile (rows 0:64 = h0, 64:128 = h1)
            ct = ctxT_pool.tile([128, CHUNK], BF16, tag=f"ctxT{j}",
                                name=f"ctxT{j}")
            ctxT[(c % 2, j)] = ct
            r0 = norm_pool.tile([1, CHUNK], F32, tag="r0", name="r0")
            r1 = norm_pool.tile([1, CHUNK], F32, tag="r1", name="r1")
            nc.vector.reciprocal(r0[:], ctx01[64:65, 0:CHUNK])
            nc.vector.reciprocal(r1[:], ctx01[64:65, CHUNK:2 * CHUNK])
            bc0 = norm_pool.tile([128, CHUNK], F32, tag="bc0", name="bc0")
            bc1 = norm_pool.tile([128, CHUNK], F32, tag="bc1", name="bc1")
            nc.gpsimd.partition_broadcast(bc0[:], r0[:])
            nc.gpsimd.partition_broadcast(bc1[:], r1[:])
            nc.vector.tensor_mul(ct[0:64, :], ctx01[0:64, 0:CHUNK],
                                 bc0[0:64, :])
            nc.vector.tensor_copy(ct[64:128, :], ctx01[0:64, CHUNK:2 * CHUNK])
            nc.vector.tensor_mul(ct[64:128, :], ct[64:128, :], bc1[64:128, :])

        def outproj_unit(c, tt):
            q0 = c * CHUNK
            for ob in range(D // 512):
                ps = proj_ps.tile([128, 512], F32, tag="proj", name="projps")
                for hdt in range(NPAIR):
                    nc.tensor.matmul(
                        ps[:],
                        ctxT[(c % 2, hdt)][:, tt * 128:(tt + 1) * 128],
                        w_sb["o"][hdt][:, ob * 512:(ob + 1) * 512],
                        start=(hdt == 0), stop=(hdt == NPAIR - 1))
                osb = out_pool.tile([128, 512], F32, tag="osb", name="osb")
                nc.vector.tensor_add(
                    osb[:], ps[:], bias_bc[:, ob * 512:(ob + 1) * 512])
                nc.sync.dma_start(
                    out_ext[q0 + tt * 128:q0 + (tt + 1) * 128,
                            ob * 512:(ob + 1) * 512],
                    osb[:])

        # ---- emission schedule ----------------------------------------
        load_weights_qkv()
        load_x(0)
        load_x(1)
        load_weights_wo()
        mask_sb = small.tile([128, 128], BF16, tag="mask")
        nc.sync.dma_start(mask_sb[:], mask_ext[:])
        bias_row = small.tile([1, D], F32, tag="biasrow")
        nc.sync.dma_start(bias_row[:], bias_ext[:])
        bias_bc = small.tile([128, D], F32, tag="biasbc")
        nc.gpsimd.partition_broadcast(bias_bc[:], bias_row[:])

        for u in qkv_units(0):
            u()

        class Filler:
            """Dispense filler units evenly across the chunk's kt stream."""

            def __init__(self, units, total_kt):
                self.units = list(units)
                self.acc = 0.0
                self.rate = len(self.units) / max(1, total_kt)

            def tick(self):
                self.acc += self.rate
                while self.acc >= 1.0 and self.units:
                    self.acc -= 1.0
                    self.units.pop(0)()

            def drain(self):
                for u in self.units:
                    u()
                self.units = []

        # per-chunk filler supply: next t-block's QKV chains + the previous
        # chunk's out-projection (both dependency-legal in this window)
        for c in range(N_CHUNK):
            if c == 1:
                load_x(2)
            if c == 2:
                load_x(3)
            units = []
            if c + 1 < NT_TB:
                units += qkv_units(c + 1)
            if c >= 1:
                units += [lambda tt=tt, cc=c - 1: outproj_unit(cc, tt)
                          for tt in range(4)]
            fill = Filler(units, NPAIR * (4 * c + 4))
            for j in range(NPAIR):
                pair_attention(j, c, fill)
            fill.drain()
        for tt in range(4):
            outproj_unit(3, tt)

    nc.compile()
    return nc


def _get_nc():
    global _NC
    if _NC is None:
        _NC = _build()
    return _NC


def _make_in_maps(x, Wq, Wk, Wv, Wo, bo):
    import ml_dtypes
    mask = np.triu(np.ones((128, 128), dtype=np.float32)).astype(
        ml_dtypes.bfloat16)
    zero_bias = np.zeros((1, D), dtype=np.float32)
    xT = [np.ascontiguousarray(x[b].T) for b in range(B)]
    in_maps = []
    for c in range(N_CORES):
        b, g = c // 2, c % 2
        sl = slice(g * HD, (g + 1) * HD)
        in_maps.append({
            "xT": xT[b].astype(ml_dtypes.bfloat16),
            "wqT": np.ascontiguousarray(Wq[sl, :].T).astype(ml_dtypes.bfloat16),
            "wkT": np.ascontiguousarray(Wk[sl, :].T).astype(ml_dtypes.bfloat16),
            "wvT": np.ascontiguousarray(Wv[sl, :].T).astype(ml_dtypes.bfloat16),
            "woT": np.ascontiguousarray(Wo[:, sl].T).astype(ml_dtypes.bfloat16),
            "bias": bo.reshape(1, D) if g == 0 else zero_bias,
            "mask": mask,
        })
    return in_maps


def kernel(x, Wq, Wk, Wv, Wo, bo):
    from concourse.bass_utils import run_bass_kernel_spmd

    x = np.asarray(x, dtype=np.float32)
    Wq = np.asarray(Wq, dtype=np.float32)
    Wk = np.asarray(Wk, dtype=np.float32)
    Wv = np.asarray(Wv, dtype=np.float32)
    Wo = np.asarray(Wo, dtype=np.float32)
    bo = np.asarray(bo, dtype=np.float32)

    nc = _get_nc()
    in_maps = _make_in_maps(x, Wq, Wk, Wv, Wo, bo)
    res = run_bass_kernel_spmd(nc, in_maps, list(range(N_CORES)))
    outs = [res.results[c]["out"] for c in range(N_CORES)]
    return np.stack([outs[2 * b] + outs[2 * b + 1] for b in range(B)], axis=0)


# revision 7
# speedup vs baseline: 1.0483x; 1.0483x over previous
"""Multi-head causal self-attention on 8 Trainium2 NeuronCores.

Sharding: 4-way data parallel over batch x 2-way tensor parallel over heads.
Core c handles batch c//2 and head group c%2 (8 of 16 heads). Each core
computes QKV projections for its head group, causal attention, and a partial
output projection (row-split Wo); the two partials per batch are summed on
the host. Bias is added on-device by the g=0 core (g=1 gets zeros).

Schedule (all bf16 matmuls, fp32 PSUM):
- Heads are processed in PAIRS (2j, 2j+1). The two K^T.Q score matmuls have
  contraction 64, so they run concurrently in PE row-groups 0/64, writing the
  two halves of one [128,1024] PSUM tile. One exp (scalar engine) covers both.
- V tiles carry a ones column, so the ctx matmul also yields the softmax
  denominators; each pair is normalized right after its ctx accumulation
  finishes (tiny [1,512] reciprocals), off the PE critical path.
- Query dim is walked in 512-wide chunks; each chunk's out-projection and the
  next t-block's QKV chains are emitted BETWEEN attention pairs so the PE
  queue never drains while the scalar engine works through the exp stream
  (keeps the PE HAM-warm at 2.4 GHz).
"""

import numpy as np

B, T, D = 4, 2048, 1024
HEADS = 16
N_CORES = 8
HPC = 8               # heads per core
HD = HPC * 64         # 512, per-core projection width
NT_D = D // 128       # 8 d-tiles
TB = 512              # t-block width for QKV streaming
NT_TB = T // TB       # 4
NT_T = T // 128       # 16 t-tiles
CHUNK = 512           # attention q-chunk width
N_CHUNK = T // CHUNK  # 4
NPAIR = HPC // 2      # 4 head pairs (pair j <-> hdt j)

_NC = None


def _build():
    import concourse.tile as tile
    import concourse.mybir as mybir
    from concourse import bacc
    from contextlib import ExitStack

    F32 = mybir.dt.float32
    BF16 = mybir.dt.bfloat16
    EXP = mybir.ActivationFunctionType.Exp

    nc = bacc.Bacc("TRN2", target_bir_lowering=False, debug=False,
                   num_devices=N_CORES)

    xT_ext = nc.dram_tensor("xT", [D, T], BF16, kind="ExternalInput")
    wqT_ext = nc.dram_tensor("wqT", [D, HD], BF16, kind="ExternalInput")
    wkT_ext = nc.dram_tensor("wkT", [D, HD], BF16, kind="ExternalInput")
    wvT_ext = nc.dram_tensor("wvT", [D, HD], BF16, kind="ExternalInput")
    woT_ext = nc.dram_tensor("woT", [HD, D], BF16, kind="ExternalInput")
    bias_ext = nc.dram_tensor("bias", [1, D], F32, kind="ExternalInput")
    mask_ext = nc.dram_tensor("mask", [128, 128], BF16, kind="ExternalInput")
    out_ext = nc.dram_tensor("out", [T, D], F32, kind="ExternalOutput")

    with tile.TileContext(nc) as tc, ExitStack() as ctx:
        # ---- pools (PSUM: 4 + 2 + 2 = 8 banks) ------------------------
        wqkv_pool = ctx.enter_context(tc.tile_pool(name="wqkv", bufs=1))
        wo_pool = ctx.enter_context(tc.tile_pool(name="wo", bufs=1))
        qk_pool = ctx.enter_context(tc.tile_pool(name="qk", bufs=1))
        v_pool = ctx.enter_context(tc.tile_pool(name="v", bufs=1))
        xt_pool = ctx.enter_context(tc.tile_pool(name="xt", bufs=2))
        small = ctx.enter_context(tc.tile_pool(name="small", bufs=1))
        pt_pool = ctx.enter_context(tc.tile_pool(name="pt", bufs=3))
        ctxT_pool = ctx.enter_context(tc.tile_pool(name="ctxT", bufs=4))
        norm_pool = ctx.enter_context(tc.tile_pool(name="norm", bufs=2))
        out_pool = ctx.enter_context(tc.tile_pool(name="outsb", bufs=3))
        s_ps_pool = ctx.enter_context(
            tc.tile_pool(name="sps", bufs=2, space="PSUM"))
        ctx_ps_pool = ctx.enter_context(
            tc.tile_pool(name="ctxps", bufs=1, space="PSUM"))
        proj_ps = ctx.enter_context(
            tc.tile_pool(name="projps", bufs=2, space="PSUM"))

        # ---- static SBUF tensors --------------------------------------
        QT = [qk_pool.tile([128, T], BF16, tag=f"QT{i}", name=f"QT{i}")
              for i in range(NPAIR)]
        KT = [qk_pool.tile([128, T], BF16, tag=f"KT{i}", name=f"KT{i}")
              for i in range(NPAIR)]
        V = [v_pool.tile([128, HPC * 65], BF16, tag=f"V{i}", name=f"V{i}")
             for i in range(NT_T)]

        w_sb = {}

        def load_weights_qkv():
            for name, ext in (("q", wqT_ext), ("k", wkT_ext), ("v", wvT_ext)):
                tiles = []
                for dt in range(NT_D):
                    t_ = wqkv_pool.tile([128, HD], BF16, tag=f"w{name}{dt}",
                                        name=f"w{name}{dt}")
                    nc.sync.dma_start(t_[:], ext[dt * 128:(dt + 1) * 128, :])
                    tiles.append(t_)
                w_sb[name] = tiles

        def load_weights_wo():
            wo_tiles = []
            for hdt in range(NPAIR):
                t_ = wo_pool.tile([128, D], BF16, tag=f"wo{hdt}",
                                  name=f"wo{hdt}")
                nc.sync.dma_start(t_[:], woT_ext[hdt * 128:(hdt + 1) * 128, :])
                wo_tiles.append(t_)
            w_sb["o"] = wo_tiles

        xts_all = {}

        def load_x(tb):
            xts = []
            for dt in range(NT_D):
                t_ = xt_pool.tile([128, TB], BF16, tag=f"xt{dt}",
                                  name=f"xt{dt}")
                nc.sync.dma_start(
                    t_[:], xT_ext[dt * 128:(dt + 1) * 128,
                                  tb * TB:(tb + 1) * TB])
                xts.append(t_)
            xts_all[tb] = xts

        # one QKV "unit" = one accumulation chain (8 matmuls) + drain copy
        def qk_unit(tb, wname, dst, hdt):
            xts = xts_all[tb]
            ps = proj_ps.tile([128, TB], F32, tag="proj", name="projps")
            for dt in range(NT_D):
                nc.tensor.matmul(
                    ps[:],
                    w_sb[wname][dt][:, hdt * 128:(hdt + 1) * 128],
                    xts[dt][:],
                    start=(dt == 0), stop=(dt == NT_D - 1))
            nc.vector.tensor_copy(
                dst[hdt][:, tb * TB:(tb + 1) * TB], ps[:])

        def v_unit(tb, j):
            xts = xts_all[tb]
            tt = tb * (TB // 128) + j
            ps = proj_ps.tile([128, HD], F32, tag="proj", name="projps")
            for dt in range(NT_D):
                nc.tensor.matmul(
                    ps[:],
                    xts[dt][:, j * 128:(j + 1) * 128],
                    w_sb["v"][dt][:],
                    start=(dt == 0), stop=(dt == NT_D - 1))
            v3 = V[tt][:].rearrange("p (h c) -> p h c", c=65)
            nc.vector.memset(v3[:, :, 64:65], 1.0)
            nc.vector.tensor_copy(
                v3[:, :, 0:64],
                ps[:].rearrange("p (h c) -> p h c", c=64))

        def qkv_units(tb):
            us = []
            for hdt in range(NPAIR):
                us.append(lambda tb=tb, hdt=hdt: qk_unit(tb, "q", QT, hdt))
            for hdt in range(NPAIR):
                us.append(lambda tb=tb, hdt=hdt: qk_unit(tb, "k", KT, hdt))
            for j in range(TB // 128):
                us.append(lambda tb=tb, j=j: v_unit(tb, j))
            return us

        # ---- attention: one head pair x one q-chunk -------------------
        ctxT = {}  # (c % 2, j) -> tile
        LAG = 2    # ctx matmuls trail the S/exp stream by this many kt

        def pair_attention(j, c, filler=None):
            q0 = c * CHUNK
            kt_max = 4 * c + 3
            h0, h1 = 2 * j, 2 * j + 1
            ctx01 = ctx_ps_pool.tile([65, 2 * CHUNK], F32, tag="ctx",
                                     name="ctxps")
            pending = []

            def emit_ctx(kt, pt):
                nc.tensor.matmul(
                    ctx01[:, 0:CHUNK],
                    V[kt][:, h0 * 65:(h0 + 1) * 65],
                    pt[:, 0:CHUNK],
                    start=(kt == 0), stop=(kt == kt_max))
                nc.tensor.matmul(
                    ctx01[:, CHUNK:2 * CHUNK],
                    V[kt][:, h1 * 65:(h1 + 1) * 65],
                    pt[:, CHUNK:2 * CHUNK],
                    start=(kt == 0), stop=(kt == kt_max))

            for kt in range(kt_max + 1):
                off = max(0, kt * 128 - q0)
                s01 = s_ps_pool.tile([128, 2 * CHUNK], F32, tag="s",
                                     name="sps")
                nc.tensor.matmul(
                    s01[:, 0:CHUNK],
                    KT[j][0:64, kt * 128:(kt + 1) * 128],
                    QT[j][0:64, q0:q0 + CHUNK],
                    start=True, stop=True)
                nc.tensor.matmul(
                    s01[:, CHUNK:2 * CHUNK],
                    KT[j][64:128, kt * 128:(kt + 1) * 128],
                    QT[j][64:128, q0:q0 + CHUNK],
                    start=True, stop=True)
                pt = pt_pool.tile([128, 2 * CHUNK], BF16, tag="pt", name="pt")
                nc.scalar.activation(pt[:, off:2 * CHUNK],
                                     s01[:, off:2 * CHUNK], EXP, scale=0.125)
                if off > 0:
                    nc.vector.memset(pt[:, 0:off], 0.0)
                    nc.vector.memset(pt[:, CHUNK:CHUNK + off], 0.0)
                if kt * 128 >= q0:
                    nc.vector.tensor_mul(pt[:, off:off + 128],
                                         pt[:, off:off + 128], mask_sb[:])
                    nc.vector.tensor_mul(
                        pt[:, CHUNK + off:CHUNK + off + 128],
                        pt[:, CHUNK + off:CHUNK + off + 128], mask_sb[:])
                pending.append((kt, pt))
                if len(pending) > LAG:
                    emit_ctx(*pending.pop(0))
                if filler is not None:
                    filler.tick()
            for item in pending:
                emit_ctx(*item)

            # normalize pair into bf16 ctxT tile (rows 0:64 = h0, 64:128 = h1)
            ct = ctxT_pool.tile([128, CHUNK], BF16, tag=f"ctxT{j}",
                                name=f"ctxT{j}")
            ctxT[(c, j)] = ct
            r0 = norm_pool.tile([1, CHUNK], F32, tag="r0", name="r0")
            r1 = norm_pool.tile([1, CHUNK], F32, tag="r1", name="r1")
            nc.vector.reciprocal(r0[:], ctx01[64:65, 0:CHUNK])
            nc.vector.reciprocal(r1[:], ctx01[64:65, CHUNK:2 * CHUNK])
            bc0 = norm_pool.tile([128, CHUNK], F32, tag="bc0", name="bc0")
            bc1 = norm_pool.tile([128, CHUNK], F32, tag="bc1", name="bc1")
            nc.gpsimd.partition_broadcast(bc0[:], r0[:])
            nc.gpsimd.partition_broadcast(bc1[:], r1[:])
            nc.vector.tensor_mul(ct[0:64, :], ctx01[0:64, 0:CHUNK],
                                 bc0[0:64, :])
            nc.vector.tensor_copy(ct[64:128, :], ctx01[0:64, CHUNK:2 * CHUNK])
            nc.vector.tensor_mul(ct[64:128, :], ct[64:128, :], bc1[64:128, :])

        def outproj_unit(c, tt):
            q0 = c * CHUNK
            for ob in range(D // 512):
                ps = proj_ps.tile([128, 512], F32, tag="proj", name="projps")
                for hdt in range(NPAIR):
                    nc.tensor.matmul(
                        ps[:],
                        ctxT[(c, hdt)][:, tt * 128:(tt + 1) * 128],
                        w_sb["o"][hdt][:, ob * 512:(ob + 1) * 512],
                        start=(hdt == 0), stop=(hdt == NPAIR - 1))
                osb = out_pool.tile([128, 512], F32, tag="osb", name="osb")
                nc.vector.tensor_add(
                    osb[:], ps[:], bias_bc[:, ob * 512:(ob + 1) * 512])
                nc.sync.dma_start(
                    out_ext[q0 + tt * 128:q0 + (tt + 1) * 128,
                            ob * 512:(ob + 1) * 512],
                    osb[:])

        # ---- emission schedule ----------------------------------------
        # interleave wq and x(tb0) DMAs so the first q-chain can start
        # as soon as possible; everything else follows
        mask_sb = small.tile([128, 128], BF16, tag="mask")
        nc.sync.dma_start(mask_sb[:], mask_ext[:])
        w_sb["q"] = []
        xts_all[0] = []
        for dt in range(NT_D):
            t_ = wqkv_pool.tile([128, HD], BF16, tag=f"wq{dt}",
                                name=f"wq{dt}")
            nc.sync.dma_start(t_[:], wqT_ext[dt * 128:(dt + 1) * 128, :])
            w_sb["q"].append(t_)
            t_ = xt_pool.tile([128, TB], BF16, tag=f"xt{dt}", name=f"xt{dt}")
            nc.sync.dma_start(t_[:], xT_ext[dt * 128:(dt + 1) * 128, 0:TB])
            xts_all[0].append(t_)
        for name, ext in (("k", wkT_ext), ("v", wvT_ext)):
            tiles = []
            for dt in range(NT_D):
                t_ = wqkv_pool.tile([128, HD], BF16, tag=f"w{name}{dt}",
                                    name=f"w{name}{dt}")
                nc.sync.dma_start(t_[:], ext[dt * 128:(dt + 1) * 128, :])
                tiles.append(t_)
            w_sb[name] = tiles
        load_x(1)
        load_weights_wo()
        bias_row = small.tile([1, D], F32, tag="biasrow")
        nc.sync.dma_start(bias_row[:], bias_ext[:])
        bias_bc = small.tile([128, D], F32, tag="biasbc")
        nc.gpsimd.partition_broadcast(bias_bc[:], bias_row[:])

        # minimal prefix of tb0 so attention pair 0 can start; the rest of
        # tb0 joins chunk 0's filler stream
        qk_unit(0, "q", QT, 0)
        qk_unit(0, "k", KT, 0)
        for jj in range(TB // 128):
            v_unit(0, jj)

        class Filler:
            """Dispense filler units across the chunk's kt stream."""

            def __init__(self, units, total_kt):
                self.units = list(units)
                self.acc = 0.0
                self.rate = len(self.units) / max(1, total_kt)

            def burst(self, n):
                for u in self.units[:n]:
                    u()
                del self.units[:n]

            def tick(self):
                self.acc += self.rate
                while self.acc >= 1.0 and self.units:
                    self.acc -= 1.0
                    self.units.pop(0)()

            def drain(self):
                for u in self.units:
                    u()
                self.units = []

        tb0_rest = []
        for hdt in range(1, NPAIR):
            tb0_rest.append(lambda hdt=hdt: qk_unit(0, "q", QT, hdt))
            tb0_rest.append(lambda hdt=hdt: qk_unit(0, "k", KT, hdt))

        # per-chunk filler supply; all out-projections are deferred to the
        # final chunk, whose causal kt range is longest but has no QKV left
        for c in range(N_CHUNK):
            if c == 1:
                load_x(2)
            if c == 2:
                load_x(3)
            units = []
            if c == 0:
                units += tb0_rest
            if c + 1 < NT_TB:
                units += qkv_units(c + 1)
            if c == 3:
                units += [lambda tt=tt, cc=cc: outproj_unit(cc, tt)
                          for cc in range(3) for tt in range(4)]
            fill = Filler(units, NPAIR * (4 * c + 4))
            for j in range(NPAIR):
                if j > 0:
                    fill.burst(2)
                pair_attention(j, c, fill)
            fill.drain()
        for tt in range(4):
            outproj_unit(3, tt)

    nc.compile()
    return nc


def _get_nc():
    global _NC
    if _NC is None:
        _NC = _build()
    return _NC


def _make_in_maps(x, Wq, Wk, Wv, Wo, bo):
    import ml_dtypes
    mask = np.triu(np.ones((128, 128), dtype=np.float32)).astype(
        ml_dtypes.bfloat16)
    zero_bias = np.zeros((1, D), dtype=np.float32)
    xT = [np.ascontiguousarray(x[b].T) for b in range(B)]
    in_maps = []
    for c in range(N_CORES):
        b, g = c // 2, c % 2
        sl = slice(g * HD, (g + 1) * HD)
        in_maps.append({
            "xT": xT[b].astype(ml_dtypes.bfloat16),
            "wqT": np.ascontiguousarray(Wq[sl, :].T).astype(ml_dtypes.bfloat16),
            "wkT": np.ascontiguousarray(Wk[sl, :].T).astype(ml_dtypes.bfloat16),
            "wvT": np.ascontiguousarray(Wv[sl, :].T).astype(ml_dtypes.bfloat16),
            "woT": np.ascontiguousarray(Wo[:, sl].T).astype(ml_dtypes.bfloat16),
            "bias": bo.reshape(1, D) if g == 0 else zero_bias,
            "mask": mask,
        })
    return in_maps


def kernel(x, Wq, Wk, Wv, Wo, bo):
    from concourse.bass_utils import run_bass_kernel_spmd

    x = np.asarray(x, dtype=np.float32)
    Wq = np.asarray(Wq, dtype=np.float32)
    Wk = np.asarray(Wk, dtype=np.float32)
    Wv = np.asarray(Wv, dtype=np.float32)
    Wo = np.asarray(Wo, dtype=np.float32)
    bo = np.asarray(bo, dtype=np.float32)

    nc = _get_nc()
    in_maps = _make_in_maps(x, Wq, Wk, Wv, Wo, bo)
    res = run_bass_kernel_spmd(nc, in_maps, list(range(N_CORES)))
    outs = [res.results[c]["out"] for c in range(N_CORES)]
    return np.stack([outs[2 * b] + outs[2 * b + 1] for b in range(B)], axis=0)


# revision 9
# speedup vs baseline: 1.3388x; 1.2771x over previous
"""Multi-head causal self-attention on 8 Trainium2 NeuronCores.

Sharding: 4-way data parallel over batch x 2-way tensor parallel over heads.
Core c handles batch c//2 and head group c%2 (8 of 16 heads). Each core
computes QKV projections for its head group, causal attention, and a partial
output projection (row-split Wo); the two partials per batch are summed on
the host. Bias is added on-device by the g=0 core (g=1 gets zeros).

Schedule (all bf16 matmuls, fp32 PSUM):
- Heads are processed in PAIRS (2j, 2j+1). The two K^T.Q score matmuls have
  contraction 64, so they run concurrently in PE row-groups 0/64, writing the
  two halves of one [128,1024] PSUM tile. One exp (scalar engine) covers both.
- V tiles carry a ones column, so the ctx matmul also yields the softmax
  denominators; each pair is normalized right after its ctx accumulation
  finishes (tiny [1,512] reciprocals), off the PE critical path.
- Query dim is walked in 512-wide chunks; each chunk's out-projection and the
  next t-block's QKV chains are emitted BETWEEN attention pairs so the PE
  queue never drains while the scalar engine works through the exp stream
  (keeps the PE HAM-warm at 2.4 GHz).
"""

import numpy as np

B, T, D = 4, 2048, 1024
HEADS = 16
N_CORES = 8
HPC = 8               # heads per core
HD = HPC * 64         # 512, per-core projection width
NT_D = D // 128       # 8 d-tiles
TB = 512              # t-block width for QKV streaming
NT_TB = T // TB       # 4
NT_T = T // 128       # 16 t-tiles
CHUNK = 512           # attention q-chunk width
N_CHUNK = T // CHUNK  # 4
NPAIR = HPC // 2      # 4 head pairs (pair j <-> hdt j)

_NC = None


def _build():
    import concourse.tile as tile
    import concourse.mybir as mybir
    from concourse import bacc
    from contextlib import ExitStack

    F32 = mybir.dt.float32
    BF16 = mybir.dt.bfloat16
    EXP = mybir.ActivationFunctionType.Exp

    nc = bacc.Bacc("TRN2", target_bir_lowering=False, debug=False,
                   num_devices=N_CORES)

    xT_ext = nc.dram_tensor("xT", [D, T], BF16, kind="ExternalInput")
    wqT_ext = nc.dram_tensor("wqT", [D, HD], BF16, kind="ExternalInput")
    wkT_ext = nc.dram_tensor("wkT", [D, HD], BF16, kind="ExternalInput")
    wvT_ext = nc.dram_tensor("wvT", [D, HD], BF16, kind="ExternalInput")
    woT_ext = nc.dram_tensor("woT", [HD, D], BF16, kind="ExternalInput")
    bias_ext = nc.dram_tensor("bias", [1, D], F32, kind="ExternalInput")
    mask_ext = nc.dram_tensor("mask", [128, 128], BF16, kind="ExternalInput")
    out_ext = nc.dram_tensor("out", [T, D], F32, kind="ExternalOutput")

    with tile.TileContext(nc) as tc, ExitStack() as ctx:
        # ---- pools (PSUM: 4 + 2 + 2 = 8 banks) ------------------------
        wqkv_pool = ctx.enter_context(tc.tile_pool(name="wqkv", bufs=1))
        wo_pool = ctx.enter_context(tc.tile_pool(name="wo", bufs=1))
        qk_pool = ctx.enter_context(tc.tile_pool(name="qk", bufs=1))
        v_pool = ctx.enter_context(tc.tile_pool(name="v", bufs=1))
        xt_pool = ctx.enter_context(tc.tile_pool(name="xt", bufs=2))
        small = ctx.enter_context(tc.tile_pool(name="small", bufs=1))
        pt_pool = ctx.enter_context(tc.tile_pool(name="pt", bufs=3))
        ctxT_pool = ctx.enter_context(tc.tile_pool(name="ctxT", bufs=4))
        norm_pool = ctx.enter_context(tc.tile_pool(name="norm", bufs=2))
        out_pool = ctx.enter_context(tc.tile_pool(name="outsb", bufs=3))
        s_ps_pool = ctx.enter_context(
            tc.tile_pool(name="sps", bufs=2, space="PSUM"))
        ctx_ps_pool = ctx.enter_context(
            tc.tile_pool(name="ctxps", bufs=1, space="PSUM"))
        proj_ps = ctx.enter_context(
            tc.tile_pool(name="projps", bufs=2, space="PSUM"))

        # ---- static SBUF tensors --------------------------------------
        QT = [qk_pool.tile([128, T], BF16, tag=f"QT{i}", name=f"QT{i}")
              for i in range(NPAIR)]
        KT = [qk_pool.tile([128, T], BF16, tag=f"KT{i}", name=f"KT{i}")
              for i in range(NPAIR)]
        V = [v_pool.tile([128, HPC * 65], BF16, tag=f"V{i}", name=f"V{i}")
             for i in range(NT_T)]

        w_sb = {}

        def load_weights_qkv():
            for name, ext in (("q", wqT_ext), ("k", wkT_ext), ("v", wvT_ext)):
                tiles = []
                for dt in range(NT_D):
                    t_ = wqkv_pool.tile([128, HD], BF16, tag=f"w{name}{dt}",
                                        name=f"w{name}{dt}")
                    nc.sync.dma_start(t_[:], ext[dt * 128:(dt + 1) * 128, :])
                    tiles.append(t_)
                w_sb[name] = tiles

        def load_weights_wo():
            wo_tiles = []
            for hdt in range(NPAIR):
                t_ = wo_pool.tile([128, D], BF16, tag=f"wo{hdt}",
                                  name=f"wo{hdt}")
                nc.sync.dma_start(t_[:], woT_ext[hdt * 128:(hdt + 1) * 128, :])
                wo_tiles.append(t_)
            w_sb["o"] = wo_tiles

        xts_all = {}

        def load_x(tb):
            xts = []
            for dt in range(NT_D):
                t_ = xt_pool.tile([128, TB], BF16, tag=f"xt{dt}",
                                  name=f"xt{dt}")
                nc.sync.dma_start(
                    t_[:], xT_ext[dt * 128:(dt + 1) * 128,
                                  tb * TB:(tb + 1) * TB])
                xts.append(t_)
            xts_all[tb] = xts

        # one QKV "unit" = one accumulation chain (8 matmuls) + drain copy
        def qk_unit(tb, wname, dst, hdt):
            xts = xts_all[tb]
            ps = proj_ps.tile([128, TB], F32, tag="proj", name="projps")
            for dt in range(NT_D):
                nc.tensor.matmul(
                    ps[:],
                    w_sb[wname][dt][:, hdt * 128:(hdt + 1) * 128],
                    xts[dt][:],
                    start=(dt == 0), stop=(dt == NT_D - 1))
            nc.vector.tensor_copy(
                dst[hdt][:, tb * TB:(tb + 1) * TB], ps[:])

        def v_unit(tb, j):
            xts = xts_all[tb]
            tt = tb * (TB // 128) + j
            ps = proj_ps.tile([128, HD], F32, tag="proj", name="projps")
            for dt in range(NT_D):
                nc.tensor.matmul(
                    ps[:],
                    xts[dt][:, j * 128:(j + 1) * 128],
                    w_sb["v"][dt][:],
                    start=(dt == 0), stop=(dt == NT_D - 1))
            v3 = V[tt][:].rearrange("p (h c) -> p h c", c=65)
            nc.vector.memset(v3[:, :, 64:65], 1.0)
            nc.vector.tensor_copy(
                v3[:, :, 0:64],
                ps[:].rearrange("p (h c) -> p h c", c=64))

        def qkv_units(tb):
            us = []
            for hdt in range(NPAIR):
                us.append(lambda tb=tb, hdt=hdt: qk_unit(tb, "q", QT, hdt))
            for hdt in range(NPAIR):
                us.append(lambda tb=tb, hdt=hdt: qk_unit(tb, "k", KT, hdt))
            for j in range(TB // 128):
                us.append(lambda tb=tb, j=j: v_unit(tb, j))
            return us

        # ---- attention: one head pair x one q-chunk -------------------
        ctxT = {}  # (c % 2, j) -> tile
        LAG = 2    # ctx matmuls trail the S/exp stream by this many kt

        def pair_attention(j, c, filler=None):
            q0 = c * CHUNK
            kt_max = 4 * c + 3
            h0, h1 = 2 * j, 2 * j + 1
            ctx01 = ctx_ps_pool.tile([65, 2 * CHUNK], F32, tag="ctx",
                                     name="ctxps")
            pending = []

            def emit_ctx(kt, pt):
                nc.tensor.matmul(
                    ctx01[:, 0:CHUNK],
                    V[kt][:, h0 * 65:(h0 + 1) * 65],
                    pt[:, 0:CHUNK],
                    start=(kt == 0), stop=(kt == kt_max))
                nc.tensor.matmul(
                    ctx01[:, CHUNK:2 * CHUNK],
                    V[kt][:, h1 * 65:(h1 + 1) * 65],
                    pt[:, CHUNK:2 * CHUNK],
                    start=(kt == 0), stop=(kt == kt_max))

            for kt in range(kt_max + 1):
                off = max(0, kt * 128 - q0)
                s01 = s_ps_pool.tile([128, 2 * CHUNK], F32, tag="s",
                                     name="sps")
                nc.tensor.matmul(
                    s01[:, 0:CHUNK],
                    KT[j][0:64, kt * 128:(kt + 1) * 128],
                    QT[j][0:64, q0:q0 + CHUNK],
                    start=True, stop=True)
                nc.tensor.matmul(
                    s01[:, CHUNK:2 * CHUNK],
                    KT[j][64:128, kt * 128:(kt + 1) * 128],
                    QT[j][64:128, q0:q0 + CHUNK],
                    start=True, stop=True)
                pt = pt_pool.tile([128, 2 * CHUNK], BF16, tag="pt", name="pt")
                nc.scalar.activation(pt[:, off:2 * CHUNK],
                                     s01[:, off:2 * CHUNK], EXP, scale=0.125)
                if off > 0:
                    nc.vector.memset(pt[:, 0:off], 0.0)
                    nc.vector.memset(pt[:, CHUNK:CHUNK + off], 0.0)
                if kt * 128 >= q0:
                    nc.vector.tensor_mul(pt[:, off:off + 128],
                                         pt[:, off:off + 128], mask_sb[:])
                    nc.vector.tensor_mul(
                        pt[:, CHUNK + off:CHUNK + off + 128],
                        pt[:, CHUNK + off:CHUNK + off + 128], mask_sb[:])
                pending.append((kt, pt))
                if len(pending) > LAG:
                    emit_ctx(*pending.pop(0))
                if filler is not None:
                    filler.tick()
            for item in pending:
                emit_ctx(*item)

            # normalize pair into bf16 ctxT tile (rows 0:64 = h0, 64:128 = h1)
            ct = ctxT_pool.tile([128, CHUNK], BF16, tag=f"ctxT{j}",
                                name=f"ctxT{j}")
            ctxT[(c, j)] = ct
            den = norm_pool.tile([1, 2 * CHUNK], F32, tag="den", name="den")
            rec = norm_pool.tile([1, 2 * CHUNK], F32, tag="rec", name="rec")
            nc.vector.tensor_copy(den[:], ctx01[64:65, :])
            nc.vector.reciprocal_approx_fast(rec[:], den[:])
            bc0 = norm_pool.tile([128, CHUNK], F32, tag="bc0", name="bc0")
            bc1 = norm_pool.tile([128, CHUNK], F32, tag="bc1", name="bc1")
            nc.gpsimd.partition_broadcast(bc0[:], rec[0:1, 0:CHUNK])
            nc.gpsimd.partition_broadcast(bc1[:], rec[0:1, CHUNK:2 * CHUNK])
            nc.vector.tensor_mul(ct[0:64, :], ctx01[0:64, 0:CHUNK],
                                 bc0[0:64, :])
            nc.vector.tensor_copy(ct[64:128, :], ctx01[0:64, CHUNK:2 * CHUNK])
            nc.vector.tensor_mul(ct[64:128, :], ct[64:128, :], bc1[64:128, :])

        def outproj_unit(c, tt):
            q0 = c * CHUNK
            for ob in range(D // 512):
                ps = proj_ps.tile([128, 512], F32, tag="proj", name="projps")
                for hdt in range(NPAIR):
                    nc.tensor.matmul(
                        ps[:],
                        ctxT[(c, hdt)][:, tt * 128:(tt + 1) * 128],
                        w_sb["o"][hdt][:, ob * 512:(ob + 1) * 512],
                        start=(hdt == 0), stop=(hdt == NPAIR - 1))
                osb = out_pool.tile([128, 512], F32, tag="osb", name="osb")
                nc.vector.tensor_add(
                    osb[:], ps[:], bias_bc[:, ob * 512:(ob + 1) * 512])
                nc.sync.dma_start(
                    out_ext[q0 + tt * 128:q0 + (tt + 1) * 128,
                            ob * 512:(ob + 1) * 512],
                    osb[:])

        # ---- emission schedule ----------------------------------------
        # interleave wq and x(tb0) DMAs so the first q-chain can start
        # as soon as possible; everything else follows
        mask_sb = small.tile([128, 128], BF16, tag="mask")
        nc.sync.dma_start(mask_sb[:], mask_ext[:])
        w_sb["q"] = []
        xts_all[0] = []
        for dt in range(NT_D):
            t_ = wqkv_pool.tile([128, HD], BF16, tag=f"wq{dt}",
                                name=f"wq{dt}")
            nc.sync.dma_start(t_[:], wqT_ext[dt * 128:(dt + 1) * 128, :])
            w_sb["q"].append(t_)
            t_ = xt_pool.tile([128, TB], BF16, tag=f"xt{dt}", name=f"xt{dt}")
            nc.sync.dma_start(t_[:], xT_ext[dt * 128:(dt + 1) * 128, 0:TB])
            xts_all[0].append(t_)
        for name, ext in (("k", wkT_ext), ("v", wvT_ext)):
            tiles = []
            for dt in range(NT_D):
                t_ = wqkv_pool.tile([128, HD], BF16, tag=f"w{name}{dt}",
                                    name=f"w{name}{dt}")
                nc.sync.dma_start(t_[:], ext[dt * 128:(dt + 1) * 128, :])
                tiles.append(t_)
            w_sb[name] = tiles
        load_x(1)
        load_weights_wo()
        bias_row = small.tile([1, D], F32, tag="biasrow")
        nc.sync.dma_start(bias_row[:], bias_ext[:])
        bias_bc = small.tile([128, D], F32, tag="biasbc")
        nc.gpsimd.partition_broadcast(bias_bc[:], bias_row[:])

        # minimal prefix of tb0 so attention pair 0 can start; the rest of
        # tb0 joins chunk 0's filler stream
        qk_unit(0, "q", QT, 0)
        qk_unit(0, "k", KT, 0)
        for jj in range(TB // 128):
            v_unit(0, jj)

        class Filler:
            """Dispense filler units across the chunk's kt stream."""

            def __init__(self, units, total_kt):
                self.units = list(units)
                self.acc = 0.0
                self.rate = len(self.units) / max(1, total_kt)

            def burst(self, n):
                for u in self.units[:n]:
                    u()
                del self.units[:n]

            def tick(self):
                self.acc += self.rate
                while self.acc >= 1.0 and self.units:
                    self.acc -= 1.0
                    self.units.pop(0)()

            def drain(self):
                for u in self.units:
                    u()
                self.units = []

        tb0_rest = []
        for hdt in range(1, NPAIR):
            tb0_rest.append(lambda hdt=hdt: qk_unit(0, "q", QT, hdt))
            tb0_rest.append(lambda hdt=hdt: qk_unit(0, "k", KT, hdt))

        # per-chunk filler supply; all out-projections are deferred to the
        # final chunk, whose causal kt range is longest but has no QKV left
        for c in range(N_CHUNK):
            if c == 1:
                load_x(2)
            if c == 2:
                load_x(3)
            units = []
            if c == 0:
                units += tb0_rest
            if c + 1 < NT_TB:
                units += qkv_units(c + 1)
            if c == 3:
                units += [lambda tt=tt, cc=cc: outproj_unit(cc, tt)
                          for cc in range(3) for tt in range(4)]
            fill = Filler(units, NPAIR * (4 * c + 4))
            for j in range(NPAIR):
                if j > 0:
                    fill.burst(2)
                pair_attention(j, c, fill)
            fill.drain()
        for tt in range(4):
            outproj_unit(3, tt)

    nc.compile()
    return nc


def _get_nc():
    global _NC
    if _NC is None:
        _NC = _build()
    return _NC


def _make_in_maps(x, Wq, Wk, Wv, Wo, bo):
    import ml_dtypes
    mask = np.triu(np.ones((128, 128), dtype=np.float32)).astype(
        ml_dtypes.bfloat16)
    zero_bias = np.zeros((1, D), dtype=np.float32)
    xT = [np.ascontiguousarray(x[b].T) for b in range(B)]
    in_maps = []
    for c in range(N_CORES):
        b, g = c // 2, c % 2
        sl = slice(g * HD, (g + 1) * HD)
        in_maps.append({
            "xT": xT[b].astype(ml_dtypes.bfloat16),
            "wqT": np.ascontiguousarray(Wq[sl, :].T).astype(ml_dtypes.bfloat16),
            "wkT": np.ascontiguousarray(Wk[sl, :].T).astype(ml_dtypes.bfloat16),
            "wvT": np.ascontiguousarray(Wv[sl, :].T).astype(ml_dtypes.bfloat16),
            "woT": np.ascontiguousarray(Wo[:, sl].T).astype(ml_dtypes.bfloat16),
            "bias": bo.reshape(1, D) if g == 0 else zero_bias,
            "mask": mask,
        })
    return in_maps


def kernel(x, Wq, Wk, Wv, Wo, bo):
    from concourse.bass_utils import run_bass_kernel_spmd

    x = np.asarray(x, dtype=np.float32)
    Wq = np.asarray(Wq, dtype=np.float32)
    Wk = np.asarray(Wk, dtype=np.float32)
    Wv = np.asarray(Wv, dtype=np.float32)
    Wo = np.asarray(Wo, dtype=np.float32)
    bo = np.asarray(bo, dtype=np.float32)

    nc = _get_nc()
    in_maps = _make_in_maps(x, Wq, Wk, Wv, Wo, bo)
    res = run_bass_kernel_spmd(nc, in_maps, list(range(N_CORES)))
    outs = [res.results[c]["out"] for c in range(N_CORES)]
    return np.stack([outs[2 * b] + outs[2 * b + 1] for b in range(B)], axis=0)
